# revision 14
# baseline (speedup 1.0000x reference)
"""HAGMoE Trainium2 kernel: hierarchical-routed 24-expert MoE, expert-parallel on 8 cores.

Reference computation (B=1024, H=768, I=3072, G=3 groups, E=8 experts/group):
    h_cond  = cat(h_fused, h_aspect) @ Wc + bc
    p_group = softmax(h_fused @ Wg + bg)
    p_exp   = softmax(h_cond @ Wr[g] + br[g])  per group
    h_moe   = sum_{g,e} p_group[:,g] * p_exp[:,g,e] * fc2(gelu(fc1(h_fused)))
    out     = h_fused + h_moe

Sharding: core c owns experts (g, c) for g=0..2 (one expert per group).  Routing
(0.1% of the FLOPs) runs on the host in fp32; each core receives its combine
weights wsel[b, g] = p_group * p_expert / S2 ready-made.  Expert matmuls run in
fp8 (e4m3) with DoubleRow perf mode (2 contraction rows per PE cell per cycle),
weights pre-scaled by S1/S2 to dodge the fp8 subnormal range; descaling is
folded into the fc1 activation (1/S1) and the host wsel (1/S2).  All device
inputs are host-prearranged into the exact SBUF layout so every DMA is a
straight [128, contiguous] copy.  Host gathers: out = h_fused + b2_term +
sum_c partial_c.
"""

import os
import sys

if "/opt/trn_rl_repo" not in sys.path:
    sys.path.insert(0, "/opt/trn_rl_repo")

import numpy as np
import ml_dtypes

B, H, I, G, E = 1024, 768, 3072, 3, 8
NCORES = 8
KH = H // 128   # 6  k-chunks for the H contraction
KI = I // 128   # 24 k-chunks for the I contraction
MI = I // 128   # 24 fc1 output chunks
MB = B // 128   # 8  token chunks
KC1 = KH // 2   # 3  DoubleRow chunks (256-deep) for fc1
KC2 = KI // 2   # 12 DoubleRow chunks (256-deep) for fc2
S1 = 32.0       # fc1 weight pre-scale (fp8 subnormal dodge)
S2 = 32.0       # fc2 weight pre-scale
F8 = ml_dtypes.float8_e4m3

_nc_cache = None


def _build_nc():
    from concourse import bacc
    import concourse.mybir as mybir
    from concourse.tile import TileContext

    dt = mybir.dt
    AF = mybir.ActivationFunctionType
    DR = mybir.MatmulPerfMode.DoubleRow

    nc = bacc.Bacc("TRN2", target_bir_lowering=False, debug=False, num_devices=NCORES)

    # ---- DRAM I/O (all host-prearranged to [128, contiguous]) ----
    xf8_d = nc.dram_tensor("xf8", [128, KH * B], dt.float8e4, kind="ExternalInput")
    w1_d = nc.dram_tensor("w1", [128, G * KH * I], dt.float8e4, kind="ExternalInput")
    b1_d = nc.dram_tensor("b1", [128, G * MI], dt.float32, kind="ExternalInput")
    w2_d = nc.dram_tensor("w2", [128, G * KI * H], dt.float8e4, kind="ExternalInput")
    wsel_d = nc.dram_tensor("wsel", [128, MB * G], dt.float32, kind="ExternalInput")
    out_d = nc.dram_tensor("out", [B, H], dt.float32, kind="ExternalOutput")

    with TileContext(nc) as tc:
        with (
            tc.tile_pool(name="xp", bufs=1) as xp,
            tc.tile_pool(name="constp", bufs=1) as constp,
            tc.tile_pool(name="w1p", bufs=2) as w1p,
            tc.tile_pool(name="w2p", bufs=2) as w2p,
            tc.tile_pool(name="h1p", bufs=2) as h1p,
            tc.tile_pool(name="accp", bufs=1) as accp,
            tc.tile_pool(name="ps1", bufs=2, space="PSUM") as ps1,
            tc.tile_pool(name="ps2", bufs=2, space="PSUM") as ps2,
        ):
            # ---- persistent tiles; DMA issue order = Sync stream order ----
            # xf8 split by DR k-pair chunk so fc1 c=0 starts after 0.26MB
            xf8t = xp.tile([128, KH * B], dt.float8e4, name="xf8t")
            nc.sync.dma_start(
                out=xf8t[:, 0 : 2 * B], in_=xf8_d.ap()[:, 0 : 2 * B]
            )
            acc = accp.tile([128, MB * H], dt.float32, name="acct")

            # PE warm-up: ~4us of junk matmuls during the initial DMA wait so
            # the HAM clock gate reaches 8/8 before the first real matmul.
            wrm = constp.tile([128, 128], dt.bfloat16, name="wrmt")
            nc.vector.memset(wrm[:], 0.0)
            pw = ps1.tile([128, 1024], dt.float32, name="pwt", tag="ps1t")
            for _ in range(34):
                nc.tensor.matmul(pw[0:128, 0:128], wrm[:], wrm[:],
                                 start=True, stop=True)

            for j in range(G):
                w1t = w1p.tile([128, KH * I], dt.float8e4, name=f"w1t{j}", tag="w1")
                if j == 0:
                    # split along i, m=22/23's slice first (they run first; see
                    # M_ORDER), then the rest of xf8 and the remaining weights
                    w1v_d = w1_d.ap()[:, j * KH * I : (j + 1) * KH * I].rearrange(
                        "p (k i) -> p k i", i=I
                    )
                    w1v_s = w1t[:].rearrange("p (k i) -> p k i", i=I)
                    sl = slice(2816, 3072)
                    nc.sync.dma_start(out=w1v_s[:, :, sl], in_=w1v_d[:, :, sl])
                    nc.sync.dma_start(
                        out=xf8t[:, 2 * B : 4 * B], in_=xf8_d.ap()[:, 2 * B : 4 * B]
                    )
                    nc.sync.dma_start(
                        out=xf8t[:, 4 * B : 6 * B], in_=xf8_d.ap()[:, 4 * B : 6 * B]
                    )
                    b1t = constp.tile([128, G * MI], dt.float32, name="b1t")
                    nc.sync.dma_start(out=b1t[:], in_=b1_d.ap())
                    sl = slice(0, 1024)
                    nc.sync.dma_start(out=w1v_s[:, :, sl], in_=w1v_d[:, :, sl])
                    wselt = constp.tile([128, MB * G], dt.float32, name="wselt")
                    nc.sync.dma_start(out=wselt[:], in_=wsel_d.ap())
                    sl = slice(1024, 2816)
                    nc.sync.dma_start(out=w1v_s[:, :, sl], in_=w1v_d[:, :, sl])
                else:
                    nc.sync.dma_start(
                        out=w1t[:], in_=w1_d.ap()[:, j * KH * I : (j + 1) * KH * I]
                    )
                w2t = w2p.tile([128, KI * H], dt.float8e4, name=f"w2t{j}", tag="w2")
                nc.sync.dma_start(
                    out=w2t[:], in_=w2_d.ap()[:, j * KI * H : (j + 1) * KI * H]
                )
                h1g = h1p.tile([128, MI * B], dt.float8e4, name=f"h1g{j}", tag="h1")

                # fc1: h1T[i, b] = gelu(sum_h W1[h,i]*x[b,h]/S1 + b1), fp8 out
                w1v = w1t[:].rearrange("p (k i) -> p k i", i=I)
                xv = xf8t[:].rearrange("p (k b) -> p k b", b=B)
                # m=22/23 first: fc2(j) t=0 consumes h1 chunk pairs in c order,
                # so the last-produced pair lands at c=10, giving the trailing
                # gelu ACT ~2us of slack instead of racing the PE at c=11.
                for m in [22, 23] + list(range(22)):
                    ps = ps1.tile([128, 1024], dt.float32, name=f"ps{j}_{m}", tag="ps1t")
                    for c in range(KC1):
                        lhs = w1v[:, 2 * c : 2 * c + 2, m * 128 : (m + 1) * 128]
                        nc.tensor.matmul(
                            ps[:, 0:512],
                            lhs,
                            xv[:, 2 * c : 2 * c + 2, 0:512],
                            start=(c == 0),
                            stop=(c == KC1 - 1),
                            perf_mode=DR,
                        )
                        nc.tensor.matmul(
                            ps[:, 512:1024],
                            lhs,
                            xv[:, 2 * c : 2 * c + 2, 512:1024],
                            start=(c == 0),
                            stop=(c == KC1 - 1),
                            perf_mode=DR,
                        )
                    bcol = b1t[:, j * MI + m : j * MI + m + 1]
                    nc.scalar.activation(
                        h1g[:, m * B : (m + 1) * B], ps[:], AF.Gelu,
                        bias=bcol, scale=1.0 / S1,
                    )

                # fc2: h2[b, h] = sum_i h1T[i,b] * W2[i,h]; weighted accumulate
                h1v = h1g[:].rearrange("p (m b) -> p m b", b=B)
                w2v = w2t[:].rearrange("p (k h) -> p k h", h=H)
                for t in range(MB):
                    p2 = ps2.tile([128, 1024], dt.float32, name=f"p2{j}_{t}", tag="p2")
                    for c in range(KC2):
                        lhs = h1v[:, 2 * c : 2 * c + 2, t * 128 : (t + 1) * 128]
                        nc.tensor.matmul(
                            p2[:, 0:512],
                            lhs,
                            w2v[:, 2 * c : 2 * c + 2, 0:512],
                            start=(c == 0),
                            stop=(c == KC2 - 1),
                            perf_mode=DR,
                        )
                        nc.tensor.matmul(
                            p2[:, 512:768],
                            lhs,
                            w2v[:, 2 * c : 2 * c + 2, 512:768],
                            start=(c == 0),
                            stop=(c == KC2 - 1),
                            perf_mode=DR,
                        )
                    wcol = wselt[:, t * G + j : t * G + j + 1]
                    if j == 0:
                        # ScalarE is idle during fc2; do the first combine there
                        # to keep the DVE queue short
                        nc.scalar.activation(
                            acc[:, t * H : (t + 1) * H], p2[:, 0:768],
                            AF.Copy, scale=wcol,
                        )
                    else:
                        # per-bank combine: each DVE op is shorter, frees the
                        # psum tile sooner, and (j==2) overlaps the store
                        for lo, hi in ((0, 512), (512, 768)):
                            nc.vector.scalar_tensor_tensor(
                                acc[:, t * H + lo : t * H + hi],
                                p2[:, lo:hi], wcol,
                                acc[:, t * H + lo : t * H + hi],
                                op0=mybir.AluOpType.mult, op1=mybir.AluOpType.add,
                            )
                            if j == G - 1:
                                nc.sync.dma_start(
                                    out=out_d.ap()[t * 128 : (t + 1) * 128, lo:hi],
                                    in_=acc[:, t * H + lo : t * H + hi],
                                )

    nc.compile()
    return nc


def _get_nc():
    global _nc_cache
    if _nc_cache is None:
        _nc_cache = _build_nc()
    return _nc_cache


def _prepare(inputs):
    h_fused = np.asarray(inputs["h_fused"], np.float32)
    h_aspect = np.asarray(inputs["h_aspect"], np.float32)
    Wc = np.asarray(inputs["Wc"], np.float32)
    bc = np.asarray(inputs["bc"], np.float32)
    Wg = np.asarray(inputs["Wg"], np.float32)
    bg = np.asarray(inputs["bg"], np.float32)
    Wr = np.asarray(inputs["Wr"], np.float32)
    br = np.asarray(inputs["br"], np.float32)
    W1 = np.asarray(inputs["W1"], np.float32)
    b1 = np.asarray(inputs["b1"], np.float32)
    W2 = np.asarray(inputs["W2"], np.float32)
    b2 = np.asarray(inputs["b2"], np.float32)

    # ---- routing on host (fp32) ----
    h_cond = np.concatenate([h_fused, h_aspect], 1) @ Wc + bc
    zg = h_fused @ Wg + bg
    zg -= zg.max(-1, keepdims=True)
    p_group = np.exp(zg)
    p_group /= p_group.sum(-1, keepdims=True)
    logits_e = np.einsum("bh,ghe->bge", h_cond, Wr) + br
    logits_e -= logits_e.max(-1, keepdims=True)
    p_expert = np.exp(logits_e)
    p_expert /= p_expert.sum(-1, keepdims=True)

    # residual + bias-2 term, added host-side after the gather
    base = h_fused + np.einsum("bg,bge,geh->bh", p_group, p_expert, b2)

    # x in SBUF layout: [p, k*B + b] = x[b, k*128 + p]
    xf8 = np.ascontiguousarray(
        h_fused.T.reshape(KH, 128, B).transpose(1, 0, 2).reshape(128, KH * B)
    ).astype(F8)

    in_maps = []
    for c in range(NCORES):
        # [G,H,I] -> [p, j*KH*I + k*I + i]
        w1c = np.ascontiguousarray(
            (W1[:, c] * S1)
            .reshape(G, KH, 128, I)
            .transpose(2, 0, 1, 3)
            .reshape(128, G * KH * I)
        ).astype(F8)
        w2c = np.ascontiguousarray(
            (W2[:, c] * S2)
            .reshape(G, KI, 128, H)
            .transpose(2, 0, 1, 3)
            .reshape(128, G * KI * H)
        ).astype(F8)
        b1c = np.ascontiguousarray(
            b1[:, c].reshape(G, MI, 128).transpose(2, 0, 1).reshape(128, G * MI)
        ).astype(np.float32)
        # wsel[p, t*G + j] = p_group[t*128+p, j] * p_expert[t*128+p, j, c] / S2
        wselc = np.ascontiguousarray(
            (p_group * p_expert[:, :, c] / S2)
            .reshape(MB, 128, G)
            .transpose(1, 0, 2)
            .reshape(128, MB * G)
        ).astype(np.float32)
        in_maps.append(
            {"xf8": xf8, "w1": w1c, "b1": b1c, "w2": w2c, "wsel": wselc}
        )

    return base, in_maps


def kernel(**inputs):
    from concourse.bass_utils import run_bass_kernel_spmd

    base, in_maps = _prepare(inputs)
    nc = _get_nc()
    res = run_bass_kernel_spmd(nc, in_maps, core_ids=list(range(NCORES)))
    out = base
    for c in range(NCORES):
        out += res.results[c]["out"]
    return out


def run_traced(**inputs):
    """Profiled run: returns BassKernelResults with exec_time_ns."""
    from concourse.bass_utils import run_bass_kernel_spmd

    base, in_maps = _prepare(inputs)
    nc = _get_nc()
    res = run_bass_kernel_spmd(nc, in_maps, core_ids=list(range(NCORES)), trace=True)
    return res


# revision 18
# speedup vs baseline: 1.0090x; 1.0090x over previous
"""HAGMoE Trainium2 kernel: hierarchical-routed 24-expert MoE, expert-parallel on 8 cores.

Reference computation (B=1024, H=768, I=3072, G=3 groups, E=8 experts/group):
    h_cond  = cat(h_fused, h_aspect) @ Wc + bc
    p_group = softmax(h_fused @ Wg + bg)
    p_exp   = softmax(h_cond @ Wr[g] + br[g])  per group
    h_moe   = sum_{g,e} p_group[:,g] * p_exp[:,g,e] * fc2(gelu(fc1(h_fused)))
    out     = h_fused + h_moe

Sharding: core c owns experts (g, c) for g=0..2 (one expert per group).  Routing
(0.1% of the FLOPs) runs on the host in fp32; each core receives its combine
weights wsel[b, g] = p_group * p_expert / S2 ready-made.  Expert matmuls run in
fp8 (e4m3) with DoubleRow perf mode (2 contraction rows per PE cell per cycle),
weights pre-scaled by S1/S2 to dodge the fp8 subnormal range; descaling is
folded into the fc1 activation (1/S1) and the host wsel (1/S2).  All device
inputs are host-prearranged into the exact SBUF layout so every DMA is a
straight [128, contiguous] copy.  Host gathers: out = h_fused + b2_term +
sum_c partial_c.
"""

import os
import sys

if "/opt/trn_rl_repo" not in sys.path:
    sys.path.insert(0, "/opt/trn_rl_repo")

import numpy as np
import ml_dtypes

B, H, I, G, E = 1024, 768, 3072, 3, 8
NCORES = 8
KH = H // 128   # 6  k-chunks for the H contraction
KI = I // 128   # 24 k-chunks for the I contraction
MI = I // 128   # 24 fc1 output chunks
MB = B // 128   # 8  token chunks
KC1 = KH // 2   # 3  DoubleRow chunks (256-deep) for fc1
KC2 = KI // 2   # 12 DoubleRow chunks (256-deep) for fc2
S1 = 32.0       # fc1 weight pre-scale (fp8 subnormal dodge)
S2 = 32.0       # fc2 weight pre-scale
F8 = ml_dtypes.float8_e4m3

_nc_cache = None


def _build_nc():
    from concourse import bacc
    import concourse.mybir as mybir
    from concourse.tile import TileContext

    dt = mybir.dt
    AF = mybir.ActivationFunctionType
    DR = mybir.MatmulPerfMode.DoubleRow

    nc = bacc.Bacc("TRN2", target_bir_lowering=False, debug=False, num_devices=NCORES)

    # ---- DRAM I/O (all host-prearranged to [128, contiguous]) ----
    xf8_d = nc.dram_tensor("xf8", [128, KH * B], dt.float8e4, kind="ExternalInput")
    w1_d = nc.dram_tensor("w1", [128, G * KH * I], dt.float8e4, kind="ExternalInput")
    b1_d = nc.dram_tensor("b1", [128, G * MI], dt.float32, kind="ExternalInput")
    w2_d = nc.dram_tensor("w2", [128, G * KI * H], dt.float8e4, kind="ExternalInput")
    wsel_d = nc.dram_tensor("wsel", [128, MB * G], dt.float32, kind="ExternalInput")
    out_d = nc.dram_tensor("out", [B, H], dt.float32, kind="ExternalOutput")

    with TileContext(nc) as tc:
        with (
            tc.tile_pool(name="xp", bufs=1) as xp,
            tc.tile_pool(name="constp", bufs=1) as constp,
            tc.tile_pool(name="w1p", bufs=2) as w1p,
            tc.tile_pool(name="w2p", bufs=2) as w2p,
            tc.tile_pool(name="h1p", bufs=2 * KC2) as h1p,
            tc.tile_pool(name="accp", bufs=1) as accp,
            tc.tile_pool(name="ps1", bufs=2, space="PSUM") as ps1,
            tc.tile_pool(name="ps2", bufs=2, space="PSUM") as ps2,
        ):
            # ---- persistent tiles; DMA issue order = Sync stream order ----
            # xf8 split by DR k-pair chunk so fc1 c=0 starts after 0.26MB
            xf8t = xp.tile([128, KH * B], dt.float8e4, name="xf8t")
            nc.sync.dma_start(
                out=xf8t[:, 0 : 2 * B], in_=xf8_d.ap()[:, 0 : 2 * B]
            )
            acc = accp.tile([128, MB * H], dt.float32, name="acct")

            # PE warm-up: ~4us of junk matmuls during the initial DMA wait so
            # the HAM clock gate reaches 8/8 before the first real matmul.
            wrm = constp.tile([128, 128], dt.bfloat16, name="wrmt")
            nc.vector.memset(wrm[:], 0.0)
            pw = ps1.tile([128, 1024], dt.float32, name="pwt", tag="ps1t")
            for _ in range(34):
                nc.tensor.matmul(pw[0:128, 0:128], wrm[:], wrm[:],
                                 start=True, stop=True)

            for j in range(G):
                w1t = w1p.tile([128, KH * I], dt.float8e4, name=f"w1t{j}", tag="w1")
                if j == 0:
                    # split along i, m=22/23's slice first (they run first; see
                    # M_ORDER), then the rest of xf8 and the remaining weights
                    w1v_d = w1_d.ap()[:, j * KH * I : (j + 1) * KH * I].rearrange(
                        "p (k i) -> p k i", i=I
                    )
                    w1v_s = w1t[:].rearrange("p (k i) -> p k i", i=I)
                    sl = slice(2816, 3072)
                    nc.sync.dma_start(out=w1v_s[:, :, sl], in_=w1v_d[:, :, sl])
                    nc.sync.dma_start(
                        out=xf8t[:, 2 * B : 4 * B], in_=xf8_d.ap()[:, 2 * B : 4 * B]
                    )
                    nc.sync.dma_start(
                        out=xf8t[:, 4 * B : 6 * B], in_=xf8_d.ap()[:, 4 * B : 6 * B]
                    )
                    b1t = constp.tile([128, G * MI], dt.float32, name="b1t")
                    nc.sync.dma_start(out=b1t[:], in_=b1_d.ap())
                    # w1 in 512-i slices, in m-consumption order, so the m-loop
                    # never outruns the serial DMA queue
                    for lo, hi in ((0, 512), (512, 1024), (1024, 1536),
                                   (1536, 2048), (2048, 2816)):
                        sl = slice(lo, hi)
                        nc.sync.dma_start(out=w1v_s[:, :, sl], in_=w1v_d[:, :, sl])
                    wselt = constp.tile([128, MB * G], dt.float32, name="wselt")
                    nc.sync.dma_start(out=wselt[:], in_=wsel_d.ap())
                else:
                    nc.sync.dma_start(
                        out=w1t[:], in_=w1_d.ap()[:, j * KH * I : (j + 1) * KH * I]
                    )
                w2t = w2p.tile([128, KI * H], dt.float8e4, name=f"w2t{j}", tag="w2")
                nc.sync.dma_start(
                    out=w2t[:], in_=w2_d.ap()[:, j * KI * H : (j + 1) * KI * H]
                )
                # one tile per DR pair-chunk: fc2's per-chunk weight load then
                # depends only on its own two gelu ACTs, not all 24
                h1ts = [
                    h1p.tile([128, 2 * B], dt.float8e4, name=f"h1_{j}_{c}", tag="h1")
                    for c in range(KC2)
                ]

                # fc1: h1T[i, b] = gelu(sum_h W1[h,i]*x[b,h]/S1 + b1), fp8 out
                w1v = w1t[:].rearrange("p (k i) -> p k i", i=I)
                xv = xf8t[:].rearrange("p (k b) -> p k b", b=B)
                # m=22/23 first: fc2(j) t=0 consumes h1 chunk pairs in c order,
                # so the last-produced pair lands at c=10, giving the trailing
                # gelu ACT ~2us of slack instead of racing the PE at c=11.
                for m in [22, 23] + list(range(22)):
                    ps = ps1.tile([128, 1024], dt.float32, name=f"ps{j}_{m}", tag="ps1t")
                    for c in range(KC1):
                        lhs = w1v[:, 2 * c : 2 * c + 2, m * 128 : (m + 1) * 128]
                        nc.tensor.matmul(
                            ps[:, 0:512],
                            lhs,
                            xv[:, 2 * c : 2 * c + 2, 0:512],
                            start=(c == 0),
                            stop=(c == KC1 - 1),
                            perf_mode=DR,
                        )
                        nc.tensor.matmul(
                            ps[:, 512:1024],
                            lhs,
                            xv[:, 2 * c : 2 * c + 2, 512:1024],
                            start=(c == 0),
                            stop=(c == KC1 - 1),
                            perf_mode=DR,
                        )
                    bcol = b1t[:, j * MI + m : j * MI + m + 1]
                    nc.scalar.activation(
                        h1ts[m // 2][:, (m % 2) * B : (m % 2 + 1) * B],
                        ps[:], AF.Gelu, bias=bcol, scale=1.0 / S1,
                    )

                # fc2: h2[b, h] = sum_i h1T[i,b] * W2[i,h]; weighted accumulate
                w2v = w2t[:].rearrange("p (k h) -> p k h", h=H)
                for t in range(MB):
                    p2 = ps2.tile([128, 1024], dt.float32, name=f"p2{j}_{t}", tag="p2")
                    for c in range(KC2):
                        lhs = h1ts[c][:].rearrange("p (two b) -> p two b", b=B)[
                            :, :, t * 128 : (t + 1) * 128
                        ]
                        nc.tensor.matmul(
                            p2[:, 0:512],
                            lhs,
                            w2v[:, 2 * c : 2 * c + 2, 0:512],
                            start=(c == 0),
                            stop=(c == KC2 - 1),
                            perf_mode=DR,
                        )
                        nc.tensor.matmul(
                            p2[:, 512:768],
                            lhs,
                            w2v[:, 2 * c : 2 * c + 2, 512:768],
                            start=(c == 0),
                            stop=(c == KC2 - 1),
                            perf_mode=DR,
                        )
                    wcol = wselt[:, t * G + j : t * G + j + 1]
                    if j == 0:
                        # ScalarE is idle during fc2; do the first combine there
                        # to keep the DVE queue short
                        nc.scalar.activation(
                            acc[:, t * H : (t + 1) * H], p2[:, 0:768],
                            AF.Copy, scale=wcol,
                        )
                    else:
                        # per-bank combine: each DVE op is shorter, frees the
                        # psum tile sooner, and (j==2) overlaps the store
                        for lo, hi in ((0, 512), (512, 768)):
                            nc.vector.scalar_tensor_tensor(
                                acc[:, t * H + lo : t * H + hi],
                                p2[:, lo:hi], wcol,
                                acc[:, t * H + lo : t * H + hi],
                                op0=mybir.AluOpType.mult, op1=mybir.AluOpType.add,
                            )
                            if j == G - 1:
                                nc.sync.dma_start(
                                    out=out_d.ap()[t * 128 : (t + 1) * 128, lo:hi],
                                    in_=acc[:, t * H + lo : t * H + hi],
                                )

    nc.compile()
    return nc


def _get_nc():
    global _nc_cache
    if _nc_cache is None:
        _nc_cache = _build_nc()
    return _nc_cache


def _prepare(inputs):
    h_fused = np.asarray(inputs["h_fused"], np.float32)
    h_aspect = np.asarray(inputs["h_aspect"], np.float32)
    Wc = np.asarray(inputs["Wc"], np.float32)
    bc = np.asarray(inputs["bc"], np.float32)
    Wg = np.asarray(inputs["Wg"], np.float32)
    bg = np.asarray(inputs["bg"], np.float32)
    Wr = np.asarray(inputs["Wr"], np.float32)
    br = np.asarray(inputs["br"], np.float32)
    W1 = np.asarray(inputs["W1"], np.float32)
    b1 = np.asarray(inputs["b1"], np.float32)
    W2 = np.asarray(inputs["W2"], np.float32)
    b2 = np.asarray(inputs["b2"], np.float32)

    # ---- routing on host (fp32) ----
    h_cond = np.concatenate([h_fused, h_aspect], 1) @ Wc + bc
    zg = h_fused @ Wg + bg
    zg -= zg.max(-1, keepdims=True)
    p_group = np.exp(zg)
    p_group /= p_group.sum(-1, keepdims=True)
    logits_e = np.einsum("bh,ghe->bge", h_cond, Wr) + br
    logits_e -= logits_e.max(-1, keepdims=True)
    p_expert = np.exp(logits_e)
    p_expert /= p_expert.sum(-1, keepdims=True)

    # residual + bias-2 term, added host-side after the gather
    base = h_fused + np.einsum("bg,bge,geh->bh", p_group, p_expert, b2)

    # x in SBUF layout: [p, k*B + b] = x[b, k*128 + p]
    xf8 = np.ascontiguousarray(
        h_fused.T.reshape(KH, 128, B).transpose(1, 0, 2).reshape(128, KH * B)
    ).astype(F8)

    in_maps = []
    for c in range(NCORES):
        # [G,H,I] -> [p, j*KH*I + k*I + i]
        w1c = np.ascontiguousarray(
            (W1[:, c] * S1)
            .reshape(G, KH, 128, I)
            .transpose(2, 0, 1, 3)
            .reshape(128, G * KH * I)
        ).astype(F8)
        w2c = np.ascontiguousarray(
            (W2[:, c] * S2)
            .reshape(G, KI, 128, H)
            .transpose(2, 0, 1, 3)
            .reshape(128, G * KI * H)
        ).astype(F8)
        b1c = np.ascontiguousarray(
            b1[:, c].reshape(G, MI, 128).transpose(2, 0, 1).reshape(128, G * MI)
        ).astype(np.float32)
        # wsel[p, t*G + j] = p_group[t*128+p, j] * p_expert[t*128+p, j, c] / S2
        wselc = np.ascontiguousarray(
            (p_group * p_expert[:, :, c] / S2)
            .reshape(MB, 128, G)
            .transpose(1, 0, 2)
            .reshape(128, MB * G)
        ).astype(np.float32)
        in_maps.append(
            {"xf8": xf8, "w1": w1c, "b1": b1c, "w2": w2c, "wsel": wselc}
        )

    return base, in_maps


def kernel(**inputs):
    from concourse.bass_utils import run_bass_kernel_spmd

    base, in_maps = _prepare(inputs)
    nc = _get_nc()
    res = run_bass_kernel_spmd(nc, in_maps, core_ids=list(range(NCORES)))
    out = base
    for c in range(NCORES):
        out += res.results[c]["out"]
    return out


def run_traced(**inputs):
    """Profiled run: returns BassKernelResults with exec_time_ns."""
    from concourse.bass_utils import run_bass_kernel_spmd

    base, in_maps = _prepare(inputs)
    nc = _get_nc()
    res = run_bass_kernel_spmd(nc, in_maps, core_ids=list(range(NCORES)), trace=True)
    return res


# revision 20
# speedup vs baseline: 1.0250x; 1.0159x over previous
"""HAGMoE Trainium2 kernel: hierarchical-routed 24-expert MoE, expert-parallel on 8 cores.

Reference computation (B=1024, H=768, I=3072, G=3 groups, E=8 experts/group):
    h_cond  = cat(h_fused, h_aspect) @ Wc + bc
    p_group = softmax(h_fused @ Wg + bg)
    p_exp   = softmax(h_cond @ Wr[g] + br[g])  per group
    h_moe   = sum_{g,e} p_group[:,g] * p_exp[:,g,e] * fc2(gelu(fc1(h_fused)))
    out     = h_fused + h_moe

Sharding: core c owns experts (g, c) for g=0..2 (one expert per group).  Routing
(0.1% of the FLOPs) runs on the host in fp32; each core receives its combine
weights wsel[b, g] = p_group * p_expert / S2 ready-made.  Expert matmuls run in
fp8 (e4m3) with DoubleRow perf mode (2 contraction rows per PE cell per cycle),
weights pre-scaled by S1/S2 to dodge the fp8 subnormal range; descaling is
folded into the fc1 activation (1/S1) and the host wsel (1/S2).  All device
inputs are host-prearranged into the exact SBUF layout so every DMA is a
straight [128, contiguous] copy.  Host gathers: out = h_fused + b2_term +
sum_c partial_c.
"""

import os
import sys

if "/opt/trn_rl_repo" not in sys.path:
    sys.path.insert(0, "/opt/trn_rl_repo")

import numpy as np
import ml_dtypes

B, H, I, G, E = 1024, 768, 3072, 3, 8
NCORES = 8
KH = H // 128   # 6  k-chunks for the H contraction
KI = I // 128   # 24 k-chunks for the I contraction
MI = I // 128   # 24 fc1 output chunks
MB = B // 128   # 8  token chunks
KC1 = KH // 2   # 3  DoubleRow chunks (256-deep) for fc1
KC2 = KI // 2   # 12 DoubleRow chunks (256-deep) for fc2
S1 = 32.0       # fc1 weight pre-scale (fp8 subnormal dodge)
S2 = 32.0       # fc2 weight pre-scale
F8 = ml_dtypes.float8_e4m3

_nc_cache = None


def _build_nc():
    from concourse import bacc
    import concourse.mybir as mybir
    from concourse.tile import TileContext

    dt = mybir.dt
    AF = mybir.ActivationFunctionType
    DR = mybir.MatmulPerfMode.DoubleRow

    nc = bacc.Bacc("TRN2", target_bir_lowering=False, debug=False, num_devices=NCORES)

    # ---- DRAM I/O (all host-prearranged to [128, contiguous]) ----
    xf8_d = nc.dram_tensor("xf8", [128, KH * B], dt.float8e4, kind="ExternalInput")
    w1_d = nc.dram_tensor("w1", [128, G * KH * I], dt.float8e4, kind="ExternalInput")
    b1_d = nc.dram_tensor("b1", [128, G * MI], dt.float32, kind="ExternalInput")
    w2_d = nc.dram_tensor("w2", [128, G * KI * H], dt.float8e4, kind="ExternalInput")
    wsel_d = nc.dram_tensor("wsel", [128, MB * G], dt.float32, kind="ExternalInput")
    out_d = nc.dram_tensor("out", [B, H], dt.float32, kind="ExternalOutput")

    with TileContext(nc) as tc:
        with (
            tc.tile_pool(name="xp", bufs=1) as xp,
            tc.tile_pool(name="constp", bufs=1) as constp,
            tc.tile_pool(name="w1p", bufs=2) as w1p,
            tc.tile_pool(name="w2p", bufs=2) as w2p,
            tc.tile_pool(name="h1p", bufs=2 * KC2) as h1p,
            tc.tile_pool(name="accp", bufs=1) as accp,
            tc.tile_pool(name="psp", bufs=4, space="PSUM") as psp,
        ):
            # ---- persistent tiles; DMA issue order = Sync stream order ----
            # xf8 split by DR k-pair chunk so fc1 c=0 starts after 0.26MB
            xf8t = xp.tile([128, KH * B], dt.float8e4, name="xf8t")
            nc.sync.dma_start(
                out=xf8t[:, 0 : 2 * B], in_=xf8_d.ap()[:, 0 : 2 * B]
            )
            acc = accp.tile([128, MB * H], dt.float32, name="acct")

            # PE warm-up: ~4us of junk matmuls during the initial DMA wait so
            # the HAM clock gate reaches 8/8 before the first real matmul.
            wrm = constp.tile([128, 128], dt.bfloat16, name="wrmt")
            nc.vector.memset(wrm[:], 0.0)
            pw = psp.tile([128, 1024], dt.float32, name="pwt", tag="ps")
            for _ in range(38):
                nc.tensor.matmul(pw[0:128, 0:128], wrm[:], wrm[:],
                                 start=True, stop=True)

            for j in range(G):
                w1t = w1p.tile([128, KH * I], dt.float8e4, name=f"w1t{j}", tag="w1")
                if j == 0:
                    # split along i, m=22/23's slice first (they run first; see
                    # M_ORDER), then the rest of xf8 and the remaining weights
                    w1v_d = w1_d.ap()[:, j * KH * I : (j + 1) * KH * I].rearrange(
                        "p (k i) -> p k i", i=I
                    )
                    w1v_s = w1t[:].rearrange("p (k i) -> p k i", i=I)
                    # interleave w1 slices with the remaining xf8 chunks in
                    # m-consumption order so the m-loop never outruns the
                    # serial DMA queue
                    sl = slice(2816, 3072)
                    nc.sync.dma_start(out=w1v_s[:, :, sl], in_=w1v_d[:, :, sl])
                    nc.sync.dma_start(
                        out=xf8t[:, 2 * B : 4 * B], in_=xf8_d.ap()[:, 2 * B : 4 * B]
                    )
                    sl = slice(0, 512)
                    nc.sync.dma_start(out=w1v_s[:, :, sl], in_=w1v_d[:, :, sl])
                    nc.sync.dma_start(
                        out=xf8t[:, 4 * B : 6 * B], in_=xf8_d.ap()[:, 4 * B : 6 * B]
                    )
                    b1t = constp.tile([128, G * MI], dt.float32, name="b1t")
                    nc.sync.dma_start(out=b1t[:], in_=b1_d.ap())
                    for lo, hi in ((512, 1024), (1024, 1536),
                                   (1536, 2048), (2048, 2816)):
                        sl = slice(lo, hi)
                        nc.sync.dma_start(out=w1v_s[:, :, sl], in_=w1v_d[:, :, sl])
                    wselt = constp.tile([128, MB * G], dt.float32, name="wselt")
                    nc.sync.dma_start(out=wselt[:], in_=wsel_d.ap())
                else:
                    nc.sync.dma_start(
                        out=w1t[:], in_=w1_d.ap()[:, j * KH * I : (j + 1) * KH * I]
                    )
                w2t = w2p.tile([128, KI * H], dt.float8e4, name=f"w2t{j}", tag="w2")
                nc.sync.dma_start(
                    out=w2t[:], in_=w2_d.ap()[:, j * KI * H : (j + 1) * KI * H]
                )
                # one tile per DR pair-chunk: fc2's per-chunk weight load then
                # depends only on its own two gelu ACTs, not all 24
                h1ts = [
                    h1p.tile([128, 2 * B], dt.float8e4, name=f"h1_{j}_{c}", tag="h1")
                    for c in range(KC2)
                ]

                # fc1: h1T[i, b] = gelu(sum_h W1[h,i]*x[b,h]/S1 + b1), fp8 out
                w1v = w1t[:].rearrange("p (k i) -> p k i", i=I)
                xv = xf8t[:].rearrange("p (k b) -> p k b", b=B)
                # m=22/23 first: fc2(j) t=0 consumes h1 chunk pairs in c order,
                # so the last-produced pair lands at c=10, giving the trailing
                # gelu ACT ~2us of slack instead of racing the PE at c=11.
                for m in [22, 23] + list(range(22)):
                    ps = psp.tile([128, 1024], dt.float32, name=f"ps{j}_{m}", tag="ps")
                    for c in range(KC1):
                        lhs = w1v[:, 2 * c : 2 * c + 2, m * 128 : (m + 1) * 128]
                        nc.tensor.matmul(
                            ps[:, 0:512],
                            lhs,
                            xv[:, 2 * c : 2 * c + 2, 0:512],
                            start=(c == 0),
                            stop=(c == KC1 - 1),
                            perf_mode=DR,
                        )
                        nc.tensor.matmul(
                            ps[:, 512:1024],
                            lhs,
                            xv[:, 2 * c : 2 * c + 2, 512:1024],
                            start=(c == 0),
                            stop=(c == KC1 - 1),
                            perf_mode=DR,
                        )
                    bcol = b1t[:, j * MI + m : j * MI + m + 1]
                    nc.scalar.activation(
                        h1ts[m // 2][:, (m % 2) * B : (m % 2 + 1) * B],
                        ps[:], AF.Gelu, bias=bcol, scale=1.0 / S1,
                    )

                # fc2: h2[b, h] = sum_i h1T[i,b] * W2[i,h]; weighted accumulate
                w2v = w2t[:].rearrange("p (k h) -> p k h", h=H)
                for t in range(MB):
                    p2 = psp.tile([128, 1024], dt.float32, name=f"p2{j}_{t}", tag="ps")
                    for c in range(KC2):
                        lhs = h1ts[c][:].rearrange("p (two b) -> p two b", b=B)[
                            :, :, t * 128 : (t + 1) * 128
                        ]
                        nc.tensor.matmul(
                            p2[:, 0:512],
                            lhs,
                            w2v[:, 2 * c : 2 * c + 2, 0:512],
                            start=(c == 0),
                            stop=(c == KC2 - 1),
                            perf_mode=DR,
                        )
                        nc.tensor.matmul(
                            p2[:, 512:768],
                            lhs,
                            w2v[:, 2 * c : 2 * c + 2, 512:768],
                            start=(c == 0),
                            stop=(c == KC2 - 1),
                            perf_mode=DR,
                        )
                    wcol = wselt[:, t * G + j : t * G + j + 1]
                    if j == 0:
                        # ScalarE is idle during fc2; do the first combine there
                        # to keep the DVE queue short
                        nc.scalar.activation(
                            acc[:, t * H : (t + 1) * H], p2[:, 0:768],
                            AF.Copy, scale=wcol,
                        )
                    else:
                        # per-bank combine: each DVE op is shorter, frees the
                        # psum tile sooner, and (j==2) overlaps the store
                        for lo, hi in ((0, 512), (512, 768)):
                            nc.vector.scalar_tensor_tensor(
                                acc[:, t * H + lo : t * H + hi],
                                p2[:, lo:hi], wcol,
                                acc[:, t * H + lo : t * H + hi],
                                op0=mybir.AluOpType.mult, op1=mybir.AluOpType.add,
                            )
                            if j == G - 1:
                                nc.sync.dma_start(
                                    out=out_d.ap()[t * 128 : (t + 1) * 128, lo:hi],
                                    in_=acc[:, t * H + lo : t * H + hi],
                                )

    nc.compile()
    return nc


def _get_nc():
    global _nc_cache
    if _nc_cache is None:
        _nc_cache = _build_nc()
    return _nc_cache


def _prepare(inputs):
    h_fused = np.asarray(inputs["h_fused"], np.float32)
    h_aspect = np.asarray(inputs["h_aspect"], np.float32)
    Wc = np.asarray(inputs["Wc"], np.float32)
    bc = np.asarray(inputs["bc"], np.float32)
    Wg = np.asarray(inputs["Wg"], np.float32)
    bg = np.asarray(inputs["bg"], np.float32)
    Wr = np.asarray(inputs["Wr"], np.float32)
    br = np.asarray(inputs["br"], np.float32)
    W1 = np.asarray(inputs["W1"], np.float32)
    b1 = np.asarray(inputs["b1"], np.float32)
    W2 = np.asarray(inputs["W2"], np.float32)
    b2 = np.asarray(inputs["b2"], np.float32)

    # ---- routing on host (fp32) ----
    h_cond = np.concatenate([h_fused, h_aspect], 1) @ Wc + bc
    zg = h_fused @ Wg + bg
    zg -= zg.max(-1, keepdims=True)
    p_group = np.exp(zg)
    p_group /= p_group.sum(-1, keepdims=True)
    logits_e = np.einsum("bh,ghe->bge", h_cond, Wr) + br
    logits_e -= logits_e.max(-1, keepdims=True)
    p_expert = np.exp(logits_e)
    p_expert /= p_expert.sum(-1, keepdims=True)

    # residual + bias-2 term, added host-side after the gather
    base = h_fused + np.einsum("bg,bge,geh->bh", p_group, p_expert, b2)

    # x in SBUF layout: [p, k*B + b] = x[b, k*128 + p]
    xf8 = np.ascontiguousarray(
        h_fused.T.reshape(KH, 128, B).transpose(1, 0, 2).reshape(128, KH * B)
    ).astype(F8)

    in_maps = []
    for c in range(NCORES):
        # [G,H,I] -> [p, j*KH*I + k*I + i]
        w1c = np.ascontiguousarray(
            (W1[:, c] * S1)
            .reshape(G, KH, 128, I)
            .transpose(2, 0, 1, 3)
            .reshape(128, G * KH * I)
        ).astype(F8)
        w2c = np.ascontiguousarray(
            (W2[:, c] * S2)
            .reshape(G, KI, 128, H)
            .transpose(2, 0, 1, 3)
            .reshape(128, G * KI * H)
        ).astype(F8)
        b1c = np.ascontiguousarray(
            b1[:, c].reshape(G, MI, 128).transpose(2, 0, 1).reshape(128, G * MI)
        ).astype(np.float32)
        # wsel[p, t*G + j] = p_group[t*128+p, j] * p_expert[t*128+p, j, c] / S2
        wselc = np.ascontiguousarray(
            (p_group * p_expert[:, :, c] / S2)
            .reshape(MB, 128, G)
            .transpose(1, 0, 2)
            .reshape(128, MB * G)
        ).astype(np.float32)
        in_maps.append(
            {"xf8": xf8, "w1": w1c, "b1": b1c, "w2": w2c, "wsel": wselc}
        )

    return base, in_maps


def kernel(**inputs):
    from concourse.bass_utils import run_bass_kernel_spmd

    base, in_maps = _prepare(inputs)
    nc = _get_nc()
    res = run_bass_kernel_spmd(nc, in_maps, core_ids=list(range(NCORES)))
    out = base
    for c in range(NCORES):
        out += res.results[c]["out"]
    return out


def run_traced(**inputs):
    """Profiled run: returns BassKernelResults with exec_time_ns."""
    from concourse.bass_utils import run_bass_kernel_spmd

    base, in_maps = _prepare(inputs)
    nc = _get_nc()
    res = run_bass_kernel_spmd(nc, in_maps, core_ids=list(range(NCORES)), trace=True)
    return res


# revision 21
# speedup vs baseline: 1.0261x; 1.0011x over previous
"""HAGMoE Trainium2 kernel: hierarchical-routed 24-expert MoE, expert-parallel on 8 cores.

Reference computation (B=1024, H=768, I=3072, G=3 groups, E=8 experts/group):
    h_cond  = cat(h_fused, h_aspect) @ Wc + bc
    p_group = softmax(h_fused @ Wg + bg)
    p_exp   = softmax(h_cond @ Wr[g] + br[g])  per group
    h_moe   = sum_{g,e} p_group[:,g] * p_exp[:,g,e] * fc2(gelu(fc1(h_fused)))
    out     = h_fused + h_moe

Sharding: core c owns experts (g, c) for g=0..2 (one expert per group).  Routing
(0.1% of the FLOPs) runs on the host in fp32; each core receives its combine
weights wsel[b, g] = p_group * p_expert / S2 ready-made.  Expert matmuls run in
fp8 (e4m3) with DoubleRow perf mode (2 contraction rows per PE cell per cycle),
weights pre-scaled by S1/S2 to dodge the fp8 subnormal range; descaling is
folded into the fc1 activation (1/S1) and the host wsel (1/S2).  All device
inputs are host-prearranged into the exact SBUF layout so every DMA is a
straight [128, contiguous] copy.  Host gathers: out = h_fused + b2_term +
sum_c partial_c.
"""

import os
import sys

if "/opt/trn_rl_repo" not in sys.path:
    sys.path.insert(0, "/opt/trn_rl_repo")

import numpy as np
import ml_dtypes

B, H, I, G, E = 1024, 768, 3072, 3, 8
NCORES = 8
KH = H // 128   # 6  k-chunks for the H contraction
KI = I // 128   # 24 k-chunks for the I contraction
MI = I // 128   # 24 fc1 output chunks
MB = B // 128   # 8  token chunks
KC1 = KH // 2   # 3  DoubleRow chunks (256-deep) for fc1
KC2 = KI // 2   # 12 DoubleRow chunks (256-deep) for fc2
S1 = 32.0       # fc1 weight pre-scale (fp8 subnormal dodge)
S2 = 32.0       # fc2 weight pre-scale
F8 = ml_dtypes.float8_e4m3

_nc_cache = None


def _build_nc():
    from concourse import bacc
    import concourse.mybir as mybir
    from concourse.tile import TileContext

    dt = mybir.dt
    AF = mybir.ActivationFunctionType
    DR = mybir.MatmulPerfMode.DoubleRow

    nc = bacc.Bacc("TRN2", target_bir_lowering=False, debug=False, num_devices=NCORES)

    # ---- DRAM I/O (all host-prearranged to [128, contiguous]) ----
    xf8_d = nc.dram_tensor("xf8", [128, KH * B], dt.float8e4, kind="ExternalInput")
    w1_d = nc.dram_tensor("w1", [128, G * KH * I], dt.float8e4, kind="ExternalInput")
    b1_d = nc.dram_tensor("b1", [128, G * MI], dt.float32, kind="ExternalInput")
    w2_d = nc.dram_tensor("w2", [128, G * KI * H], dt.float8e4, kind="ExternalInput")
    wsel_d = nc.dram_tensor("wsel", [128, MB * G], dt.float32, kind="ExternalInput")
    out_d = nc.dram_tensor("out", [B, H], dt.float32, kind="ExternalOutput")

    with TileContext(nc) as tc:
        with (
            tc.tile_pool(name="sb", bufs=1) as sb,
            tc.tile_pool(name="wp", bufs=2) as wp,
            tc.tile_pool(name="h1p", bufs=2 * KC2) as h1p,
            tc.tile_pool(name="psp", bufs=4, space="PSUM") as psp,
        ):
            # ---- persistent tiles; DMA issue order = Sync stream order ----
            # xf8 split by DR k-pair chunk so fc1 c=0 starts after 0.26MB
            xf8t = sb.tile([128, KH * B], dt.float8e4, name="xf8t")
            nc.sync.dma_start(
                out=xf8t[:, 0 : 2 * B], in_=xf8_d.ap()[:, 0 : 2 * B]
            )
            acc = sb.tile([128, MB * H], dt.float32, name="acct")

            # PE warm-up: ~4us of junk matmuls during the initial DMA wait so
            # the HAM clock gate reaches 8/8 before the first real matmul.
            wrm = sb.tile([128, 128], dt.bfloat16, name="wrmt")
            nc.gpsimd.memset(wrm[:], 0.0)
            pw = psp.tile([128, 1024], dt.float32, name="pwt", tag="ps")
            for _ in range(36):
                nc.tensor.matmul(pw[0:128, 0:128], wrm[:], wrm[:],
                                 start=True, stop=True)

            for j in range(G):
                w1t = wp.tile([128, KH * I], dt.float8e4, name=f"w1t{j}", tag="w1")
                if j == 0:
                    # split along i, m=22/23's slice first (they run first; see
                    # M_ORDER), then the rest of xf8 and the remaining weights
                    w1v_d = w1_d.ap()[:, j * KH * I : (j + 1) * KH * I].rearrange(
                        "p (k i) -> p k i", i=I
                    )
                    w1v_s = w1t[:].rearrange("p (k i) -> p k i", i=I)
                    # interleave w1 slices with the remaining xf8 chunks in
                    # m-consumption order so the m-loop never outruns the
                    # serial DMA queue
                    sl = slice(2816, 3072)
                    nc.sync.dma_start(out=w1v_s[:, :, sl], in_=w1v_d[:, :, sl])
                    nc.sync.dma_start(
                        out=xf8t[:, 2 * B : 4 * B], in_=xf8_d.ap()[:, 2 * B : 4 * B]
                    )
                    sl = slice(0, 512)
                    nc.sync.dma_start(out=w1v_s[:, :, sl], in_=w1v_d[:, :, sl])
                    nc.sync.dma_start(
                        out=xf8t[:, 4 * B : 6 * B], in_=xf8_d.ap()[:, 4 * B : 6 * B]
                    )
                    b1t = sb.tile([128, G * MI], dt.float32, name="b1t")
                    nc.sync.dma_start(out=b1t[:], in_=b1_d.ap())
                    for lo, hi in ((512, 1024), (1024, 1536),
                                   (1536, 2048), (2048, 2816)):
                        sl = slice(lo, hi)
                        nc.sync.dma_start(out=w1v_s[:, :, sl], in_=w1v_d[:, :, sl])
                    wselt = sb.tile([128, MB * G], dt.float32, name="wselt")
                    nc.sync.dma_start(out=wselt[:], in_=wsel_d.ap())
                else:
                    nc.sync.dma_start(
                        out=w1t[:], in_=w1_d.ap()[:, j * KH * I : (j + 1) * KH * I]
                    )
                w2t = wp.tile([128, KI * H], dt.float8e4, name=f"w2t{j}", tag="w2")
                nc.sync.dma_start(
                    out=w2t[:], in_=w2_d.ap()[:, j * KI * H : (j + 1) * KI * H]
                )
                # one tile per DR pair-chunk: fc2's per-chunk weight load then
                # depends only on its own two gelu ACTs, not all 24
                h1ts = [
                    h1p.tile([128, 2 * B], dt.float8e4, name=f"h1_{j}_{c}", tag="h1")
                    for c in range(KC2)
                ]

                # fc1: h1T[i, b] = gelu(sum_h W1[h,i]*x[b,h]/S1 + b1), fp8 out
                w1v = w1t[:].rearrange("p (k i) -> p k i", i=I)
                xv = xf8t[:].rearrange("p (k b) -> p k b", b=B)
                # m=22/23 first: fc2(j) t=0 consumes h1 chunk pairs in c order,
                # so the last-produced pair lands at c=10, giving the trailing
                # gelu ACT ~2us of slack instead of racing the PE at c=11.
                for m in [22, 23] + list(range(22)):
                    ps = psp.tile([128, 1024], dt.float32, name=f"ps{j}_{m}", tag="ps")
                    for c in range(KC1):
                        lhs = w1v[:, 2 * c : 2 * c + 2, m * 128 : (m + 1) * 128]
                        nc.tensor.matmul(
                            ps[:, 0:512],
                            lhs,
                            xv[:, 2 * c : 2 * c + 2, 0:512],
                            start=(c == 0),
                            stop=(c == KC1 - 1),
                            perf_mode=DR,
                        )
                        nc.tensor.matmul(
                            ps[:, 512:1024],
                            lhs,
                            xv[:, 2 * c : 2 * c + 2, 512:1024],
                            start=(c == 0),
                            stop=(c == KC1 - 1),
                            perf_mode=DR,
                        )
                    bcol = b1t[:, j * MI + m : j * MI + m + 1]
                    nc.scalar.activation(
                        h1ts[m // 2][:, (m % 2) * B : (m % 2 + 1) * B],
                        ps[:], AF.Gelu, bias=bcol, scale=1.0 / S1,
                    )

                # fc2: h2[b, h] = sum_i h1T[i,b] * W2[i,h]; weighted accumulate
                w2v = w2t[:].rearrange("p (k h) -> p k h", h=H)
                for t in range(MB):
                    p2 = psp.tile([128, 1024], dt.float32, name=f"p2{j}_{t}", tag="ps")
                    for c in range(KC2):
                        lhs = h1ts[c][:].rearrange("p (two b) -> p two b", b=B)[
                            :, :, t * 128 : (t + 1) * 128
                        ]
                        nc.tensor.matmul(
                            p2[:, 0:512],
                            lhs,
                            w2v[:, 2 * c : 2 * c + 2, 0:512],
                            start=(c == 0),
                            stop=(c == KC2 - 1),
                            perf_mode=DR,
                        )
                        nc.tensor.matmul(
                            p2[:, 512:768],
                            lhs,
                            w2v[:, 2 * c : 2 * c + 2, 512:768],
                            start=(c == 0),
                            stop=(c == KC2 - 1),
                            perf_mode=DR,
                        )
                    wcol = wselt[:, t * G + j : t * G + j + 1]
                    if j == 0:
                        # ScalarE is idle during fc2; do the first combine there
                        # to keep the DVE queue short
                        nc.scalar.activation(
                            acc[:, t * H : (t + 1) * H], p2[:, 0:768],
                            AF.Copy, scale=wcol,
                        )
                    else:
                        # per-bank combine: each DVE op is shorter, frees the
                        # psum tile sooner, and (j==2) overlaps the store
                        for lo, hi in ((0, 512), (512, 768)):
                            nc.vector.scalar_tensor_tensor(
                                acc[:, t * H + lo : t * H + hi],
                                p2[:, lo:hi], wcol,
                                acc[:, t * H + lo : t * H + hi],
                                op0=mybir.AluOpType.mult, op1=mybir.AluOpType.add,
                            )
                            if j == G - 1:
                                nc.sync.dma_start(
                                    out=out_d.ap()[t * 128 : (t + 1) * 128, lo:hi],
                                    in_=acc[:, t * H + lo : t * H + hi],
                                )

    nc.compile()
    return nc


def _get_nc():
    global _nc_cache
    if _nc_cache is None:
        _nc_cache = _build_nc()
    return _nc_cache


def _prepare(inputs):
    h_fused = np.asarray(inputs["h_fused"], np.float32)
    h_aspect = np.asarray(inputs["h_aspect"], np.float32)
    Wc = np.asarray(inputs["Wc"], np.float32)
    bc = np.asarray(inputs["bc"], np.float32)
    Wg = np.asarray(inputs["Wg"], np.float32)
    bg = np.asarray(inputs["bg"], np.float32)
    Wr = np.asarray(inputs["Wr"], np.float32)
    br = np.asarray(inputs["br"], np.float32)
    W1 = np.asarray(inputs["W1"], np.float32)
    b1 = np.asarray(inputs["b1"], np.float32)
    W2 = np.asarray(inputs["W2"], np.float32)
    b2 = np.asarray(inputs["b2"], np.float32)

    # ---- routing on host (fp32) ----
    h_cond = np.concatenate([h_fused, h_aspect], 1) @ Wc + bc
    zg = h_fused @ Wg + bg
    zg -= zg.max(-1, keepdims=True)
    p_group = np.exp(zg)
    p_group /= p_group.sum(-1, keepdims=True)
    logits_e = np.einsum("bh,ghe->bge", h_cond, Wr) + br
    logits_e -= logits_e.max(-1, keepdims=True)
    p_expert = np.exp(logits_e)
    p_expert /= p_expert.sum(-1, keepdims=True)

    # residual + bias-2 term, added host-side after the gather
    base = h_fused + np.einsum("bg,bge,geh->bh", p_group, p_expert, b2)

    # x in SBUF layout: [p, k*B + b] = x[b, k*128 + p]
    xf8 = np.ascontiguousarray(
        h_fused.T.reshape(KH, 128, B).transpose(1, 0, 2).reshape(128, KH * B)
    ).astype(F8)

    in_maps = []
    for c in range(NCORES):
        # [G,H,I] -> [p, j*KH*I + k*I + i]
        w1c = np.ascontiguousarray(
            (W1[:, c] * S1)
            .reshape(G, KH, 128, I)
            .transpose(2, 0, 1, 3)
            .reshape(128, G * KH * I)
        ).astype(F8)
        w2c = np.ascontiguousarray(
            (W2[:, c] * S2)
            .reshape(G, KI, 128, H)
            .transpose(2, 0, 1, 3)
            .reshape(128, G * KI * H)
        ).astype(F8)
        b1c = np.ascontiguousarray(
            b1[:, c].reshape(G, MI, 128).transpose(2, 0, 1).reshape(128, G * MI)
        ).astype(np.float32)
        # wsel[p, t*G + j] = p_group[t*128+p, j] * p_expert[t*128+p, j, c] / S2
        wselc = np.ascontiguousarray(
            (p_group * p_expert[:, :, c] / S2)
            .reshape(MB, 128, G)
            .transpose(1, 0, 2)
            .reshape(128, MB * G)
        ).astype(np.float32)
        in_maps.append(
            {"xf8": xf8, "w1": w1c, "b1": b1c, "w2": w2c, "wsel": wselc}
        )

    return base, in_maps


def kernel(**inputs):
    from concourse.bass_utils import run_bass_kernel_spmd

    base, in_maps = _prepare(inputs)
    nc = _get_nc()
    res = run_bass_kernel_spmd(nc, in_maps, core_ids=list(range(NCORES)))
    out = base
    for c in range(NCORES):
        out += res.results[c]["out"]
    return out


def run_traced(**inputs):
    """Profiled run: returns BassKernelResults with exec_time_ns."""
    from concourse.bass_utils import run_bass_kernel_spmd

    base, in_maps = _prepare(inputs)
    nc = _get_nc()
    res = run_bass_kernel_spmd(nc, in_maps, core_ids=list(range(NCORES)), trace=True)
    return res
